# revision 6
# baseline (speedup 1.0000x reference)
"""Trainium2 Bass kernel for nn_AttentionD8 (dense transformer attention, D8 irreps).

Sharding: data-parallel over batch B=8 -> 8 NeuronCores, one batch element per
core. No collectives. Each core runs the full per-batch attention block.

Device layout: channel-major ("transposed") everywhere. The host pre-transposes
inputs/weights with numpy and post-transposes outputs, so the device never
transposes anything:
  - qkv projections compute y.T = Wq @ x.T directly (contraction over input
    channels on partitions).
  - attention computes S.T (keys j on partitions, queries i on free axis);
    exp(S.T) is then exactly the stationary operand of the A@V matmul, and an
    extra ones-column appended to V yields the softmax denominators for free.
  - softmax normalization is applied after attention: reciprocal of the
    denominators, PE-broadcast to (96 x 1024) tiles via tiny 0/1 selector
    matmuls (fp32), one elementwise multiply per assembled output tile.
  - biases enter via a ones-row appended to x.T and a bias-row appended to the
    weight (bf16), except the output bias bp_A1 which is added in fp32 on the
    PSUM eviction.
Matmul operands are bf16 (full-rate PE path); all accumulation is fp32 in
PSUM, softmax statistics and normalization are fp32.
"""

import os
import sys

import numpy as np

for _p in ("/opt/trn_rl_repo", os.path.expanduser("~/.axon_site/_ro/trn_rl_repo")):
    if _p not in sys.path and os.path.isdir(_p):
        sys.path.append(_p)

import concourse.bass as bass  # noqa: F401
import concourse.tile as tile
from concourse import bacc, mybir
from concourse.bass_utils import run_bass_kernel_spmd

F32 = mybir.dt.float32
BF16 = mybir.dt.bfloat16
EXP = mybir.ActivationFunctionType.Exp

B, N, H, C = 8, 1024, 12, 96
HD = 64  # head dim
NC2 = (slice(0, 512), slice(512, 1024))  # 512-token chunks


def build():
    nc = bacc.Bacc("TRN2", target_bir_lowering=False, debug=False, num_devices=8)

    def inp(name, shape, dt=BF16):
        return nc.dram_tensor(name, list(shape), dt, kind="ExternalInput").ap()

    def outp(name, shape):
        return nc.dram_tensor(name, list(shape), F32, kind="ExternalOutput").ap()

    xa = [inp("xa1t", (97, N)), inp("xa2t", (96, N)), inp("xb1t", (96, N)),
          inp("xb2t", (96, N))]
    x2 = {(r, k): inp(f"x2t_{r}_{k}", (96, N)) for r in range(2) for k in range(2)}
    wq = [inp("wqa1t", (97, 288)), inp("wqa2t", (96, 288)), inp("wqb1t", (96, 288)),
          inp("wqb2t", (96, 288))]
    wqe = [inp("wqet_0", (96, 576)), inp("wqet_1", (96, 576))]
    wp = [inp("wpa1t", (96, 96)), inp("wpa2t", (96, 96)), inp("wpb1t", (96, 96)),
          inp("wpb2t", (96, 96))]
    wpe = [inp("wpet_0", (96, 192)), inp("wpet_1", (96, 192))]
    eb1d = inp("eb1", (12, 96), F32)
    ebed = inp("ebe", (12, 192), F32)
    bpa1d = inp("bpa1", (96, 1), F32)

    zd = [outp("z1t", (96, N)), outp("z2t", (96, N)), outp("z3t", (96, N)),
          outp("z4t", (96, N))]
    zed = [outp("zet_0", (192, N)), outp("zet_1", (192, N))]

    with tile.TileContext(nc) as tc, \
         tc.tile_pool(name="w", bufs=1) as wpool, \
         tc.tile_pool(name="pack", bufs=1) as packpool:
        # ---- constant / weight loads ----
        wqs = [wpool.tile([t.shape[0], 288], BF16, name=f"wq{i}")
               for i, t in enumerate(wq)]
        wqes = [wpool.tile([96, 576], BF16, name=f"wqe{k}") for k in range(2)]
        wps = [wpool.tile([96, 96], BF16, name=f"wp{i}") for i in range(4)]
        wpes = [wpool.tile([96, 192], BF16, name=f"wpe{k}") for k in range(2)]
        for t, d in zip(wqs + wqes + wps + wpes, wq + wqe + wp + wpe):
            nc.sync.dma_start(t[:], d[:])
        eb1 = wpool.tile([12, 96], F32)
        nc.sync.dma_start(eb1[:], eb1d[:])
        ebe = wpool.tile([12, 192], F32)
        nc.sync.dma_start(ebe[:], ebed[:])
        bpa1 = wpool.tile([96, 1], F32)
        nc.sync.dma_start(bpa1[:], bpa1d[:])

        qpack = [packpool.tile([128, N], BF16, name=f"qp{i}") for i in range(6)]
        kpack = [packpool.tile([128, N], BF16, name=f"kp{i}") for i in range(6)]
        vpack = [packpool.tile([128, 12 * 65], BF16, name=f"vp{i}") for i in range(8)]

        with tc.tile_pool(name="x", bufs=1) as xpool, \
             tc.tile_pool(name="y", bufs=1) as ypool, \
             tc.tile_pool(name="pj", bufs=2, space="PSUM") as pj:
            xs = [xpool.tile([t.shape[0], N], BF16, name=f"x{i}")
                  for i, t in enumerate(xa)]
            for t, d in zip(xs, xa):
                nc.sync.dma_start(t[:], d[:])
            x2s = {}
            for r in range(2):
                for k in range(2):
                    x2s[r, k] = xpool.tile([96, N], BF16, name=f"x2{r}{k}")
                    nc.sync.dma_start(x2s[r, k][:], x2[r, k][:])

            # ---- phase A: q/k projections (channel-major) ----
            yqk = {}
            for bi in range(4):
                for qk in range(2):
                    yt = ypool.tile([96, N], BF16, name=f"y{bi}{qk}")
                    for ic in range(2):
                        ps = pj.tile([96, 512], F32, tag="pj", name="ps_pj")
                        nc.tensor.matmul(
                            ps[:],
                            wqs[bi][:, qk * 96:(qk + 1) * 96],
                            xs[bi][:, NC2[ic]],
                            start=True, stop=True,
                        )
                        nc.vector.tensor_copy(yt[:, NC2[ic]], ps[:])
                    yqk[bi, qk] = yt
            ye = {}
            for r in range(2):
                for oc in range(4):  # q0,q1,k0,k1 96-row chunks of yE
                    yt = ypool.tile([96, N], BF16, name=f"ye{r}{oc}")
                    for ic in range(2):
                        ps = pj.tile([96, 512], F32, tag="pj", name="ps_pj")
                        for k in range(2):
                            nc.tensor.matmul(
                                ps[:],
                                wqes[k][:, oc * 96:(oc + 1) * 96],
                                x2s[r, k][:, NC2[ic]],
                                start=(k == 0), stop=(k == 1),
                            )
                        nc.vector.tensor_copy(yt[:, NC2[ic]], ps[:])
                    ye[r, oc] = yt

            # ---- phase B: assemble per-head-pair q/k packs (SBUF->SBUF DMA) ----
            for h in range(H):
                hp, p = divmod(h, 2)
                for qk, pack in ((0, qpack), (1, kpack)):
                    base = p * 64
                    for bi in range(4):
                        nc.sync.dma_start(
                            pack[hp][base + bi * 8: base + (bi + 1) * 8, :],
                            yqk[bi, qk][h * 8:(h + 1) * 8, :],
                        )
                    for r in range(2):
                        oc = qk * 2 + (h // 6)
                        nc.sync.dma_start(
                            pack[hp][base + 32 + r * 16: base + 32 + (r + 1) * 16, :],
                            ye[r, oc][(h % 6) * 16:(h % 6 + 1) * 16, :],
                        )

            # ---- phase C: v projections, token-major, into packed v tiles ----
            voff = (0, 8, 16, 24)
            for t8 in range(8):
                tok = slice(t8 * 128, (t8 + 1) * 128)
                v3 = vpack[t8].rearrange("p (h c) -> p h c", c=65)
                for bi in range(4):
                    ps = pj.tile([128, 96], F32, tag="pjv", name="ps_pjv")
                    nc.tensor.matmul(
                        ps[:], xs[bi][:, tok], wqs[bi][:, 192:288],
                        start=True, stop=True,
                    )
                    nc.vector.tensor_copy(
                        v3[:, :, voff[bi]:voff[bi] + 8],
                        ps.rearrange("p (h c) -> p h c", c=8),
                    )
                for r in range(2):
                    ps = pj.tile([128, 192], F32, tag="pjv", name="ps_pjv")
                    for k in range(2):
                        nc.tensor.matmul(
                            ps[:], x2s[r, k][:, tok], wqes[k][:, 384:576],
                            start=(k == 0), stop=(k == 1),
                        )
                    nc.vector.tensor_copy(
                        v3[:, :, 32 + r * 16:32 + (r + 1) * 16],
                        ps.rearrange("p (h c) -> p h c", c=16),
                    )
                nc.vector.memset(v3[:, :, 64:65], 1.0)

        # ---- phases D/E: attention, normalize, output projections ----
        with tc.tile_pool(name="o", bufs=1) as opool:
            ots = [opool.tile([96, N], F32, name=f"o{i}t") for i in range(4)]
            oet = {(r, k): opool.tile([96, N], F32, name=f"oet{r}{k}")
                   for r in range(2) for k in range(2)}
            rin = opool.tile([12, N], F32)

            with tc.tile_pool(name="es", bufs=12) as espool, \
                 tc.tile_pool(name="avsp", bufs=3) as avspool, \
                 tc.tile_pool(name="st", bufs=2, space="PSUM") as stp, \
                 tc.tile_pool(name="av", bufs=2, space="PSUM") as avp:
                for h in range(H):
                    hp, p = divmod(h, 2)
                    base = p * 64
                    avps = avp.tile([65, N], F32, tag="av", name="ps_av")
                    for jc in range(8):
                        stps = stp.tile([128, N], F32, tag="st", name="ps_st")
                        for ic in range(2):
                            nc.tensor.matmul(
                                stps[:, NC2[ic]],
                                kpack[hp][base:base + 64, jc * 128:(jc + 1) * 128],
                                qpack[hp][base:base + 64, NC2[ic]],
                                start=True, stop=True,
                            )
                        es = espool.tile([128, N], BF16, tag="es", name="es")
                        nc.scalar.activation(es[:], stps[:], EXP,
                                             scale=float(HD) ** -0.5)
                        for ic in range(2):
                            nc.tensor.matmul(
                                avps[:, NC2[ic]],
                                vpack[jc][:, h * 65:(h + 1) * 65],
                                es[:, NC2[ic]],
                                start=(jc == 0), stop=(jc == 7),
                            )
                    avs = avspool.tile([65, N], F32, tag="avs", name="avs")
                    nc.vector.tensor_copy(avs[:], avps[:])
                    for bi in range(4):
                        nc.sync.dma_start(ots[bi][h * 8:(h + 1) * 8, :],
                                          avs[bi * 8:(bi + 1) * 8, :])
                    for r in range(2):
                        nc.sync.dma_start(
                            oet[r, h // 6][(h % 6) * 16:(h % 6 + 1) * 16, :],
                            avs[32 + r * 16:32 + (r + 1) * 16, :],
                        )
                    nc.sync.dma_start(rin[h:h + 1, :], avs[64:65, :])

            # ---- phase E: normalize (fp32) + output projections (bf16) ----
            with tc.tile_pool(name="on", bufs=1) as onpool, \
                 tc.tile_pool(name="z", bufs=4) as zpool, \
                 tc.tile_pool(name="ep", bufs=2, space="PSUM") as epp:
                otn = [onpool.tile([96, N], BF16, name=f"on{i}") for i in range(4)]
                oetn = {(r, k): onpool.tile([96, N], BF16, name=f"oen{r}{k}")
                        for r in range(2) for k in range(2)}
                rec = opool.tile([12, N], F32)
                nc.vector.reciprocal(rec[:], rin[:])
                bc1 = epp.tile([96, N], F32, tag="bc", name="ps_bc")
                for ic in range(2):
                    nc.tensor.matmul(bc1[:, NC2[ic]], eb1[:], rec[:, NC2[ic]],
                                     start=True, stop=True)
                for bi in range(4):
                    nc.vector.tensor_mul(otn[bi][:], ots[bi][:], bc1[:])
                for k in range(2):
                    bce = epp.tile([96, N], F32, tag="bc", name="ps_bc")
                    for ic in range(2):
                        nc.tensor.matmul(bce[:, NC2[ic]],
                                         ebe[:, k * 96:(k + 1) * 96],
                                         rec[:, NC2[ic]], start=True, stop=True)
                    for r in range(2):
                        nc.vector.tensor_mul(oetn[r, k][:], oet[r, k][:], bce[:])

                for zi in range(4):
                    zs = zpool.tile([96, N], F32, tag="z", name="zs")
                    for ic in range(2):
                        ps = epp.tile([96, 512], F32, tag="zps", name="ps_z")
                        nc.tensor.matmul(ps[:], wps[zi][:], otn[zi][:, NC2[ic]],
                                         start=True, stop=True)
                        if zi == 0:
                            nc.vector.tensor_scalar_add(zs[:, NC2[ic]], ps[:],
                                                        bpa1[:])
                        else:
                            nc.vector.tensor_copy(zs[:, NC2[ic]], ps[:])
                    nc.sync.dma_start(zd[zi][:], zs[:])
                for r in range(2):
                    for mc in range(2):
                        zs = zpool.tile([96, N], F32, tag="z", name="zs")
                        for ic in range(2):
                            ps = epp.tile([96, 512], F32, tag="zps", name="ps_z")
                            for k in range(2):
                                nc.tensor.matmul(
                                    ps[:], wpes[k][:, mc * 96:(mc + 1) * 96],
                                    oetn[r, k][:, NC2[ic]],
                                    start=(k == 0), stop=(k == 1),
                                )
                            nc.vector.tensor_copy(zs[:, NC2[ic]], ps[:])
                        nc.sync.dma_start(zed[r][mc * 96:(mc + 1) * 96, :], zs[:])

    nc.compile()
    return nc


def make_in_maps(inputs):
    from ml_dtypes import bfloat16

    b16 = lambda a: np.ascontiguousarray(np.asarray(a, dtype=np.float32)).astype(bfloat16)  # noqa: E731
    f32c = lambda a: np.ascontiguousarray(a, dtype=np.float32)  # noqa: E731
    ones = np.ones((1, N), np.float32)
    shared = {
        "wqa1t": b16(np.concatenate(
            [np.asarray(inputs["wq_A1"], np.float32).T,
             np.asarray(inputs["bq_A1"], np.float32)[None, :]], 0)),
        "wqa2t": b16(np.asarray(inputs["wq_A2"]).T),
        "wqb1t": b16(np.asarray(inputs["wq_B1"]).T),
        "wqb2t": b16(np.asarray(inputs["wq_B2"]).T),
        "wpa1t": b16(np.asarray(inputs["wp_A1"]).T),
        "wpa2t": b16(np.asarray(inputs["wp_A2"]).T),
        "wpb1t": b16(np.asarray(inputs["wp_B1"]).T),
        "wpb2t": b16(np.asarray(inputs["wp_B2"]).T),
        "eb1": (np.arange(96)[None, :] // 8 == np.arange(12)[:, None]).astype(np.float32),
        "ebe": (np.arange(192)[None, :] // 16 == np.arange(12)[:, None]).astype(np.float32),
        "bpa1": f32c(np.asarray(inputs["bp_A1"], np.float32)[:, None]),
    }
    for k in range(2):
        shared[f"wqet_{k}"] = b16(np.asarray(inputs["wq_E"]).T[k * 96:(k + 1) * 96])
        shared[f"wpet_{k}"] = b16(np.asarray(inputs["wp_E"]).T[k * 96:(k + 1) * 96])
    maps = []
    for b in range(B):
        m = dict(shared)
        m["xa1t"] = b16(np.concatenate(
            [np.asarray(inputs["x_A1"][b], np.float32).T, ones], 0))
        m["xa2t"] = b16(np.asarray(inputs["x_A2"][b]).T)
        m["xb1t"] = b16(np.asarray(inputs["x_B1"][b]).T)
        m["xb2t"] = b16(np.asarray(inputs["x_B2"][b]).T)
        for r in range(2):
            for k in range(2):
                m[f"x2t_{r}_{k}"] = b16(
                    np.asarray(inputs["x_2d"][b, :, r, k * 96:(k + 1) * 96]).T)
        maps.append(m)
    return maps


def assemble_outputs(results):
    z = [np.empty((B, N, 96), np.float32) for _ in range(4)]
    ze = np.empty((B, N, 2, 192), np.float32)
    for b in range(B):
        for i in range(4):
            z[i][b] = results[b][f"z{i + 1}t"].T
        for r in range(2):
            ze[b, :, r, :] = results[b][f"zet_{r}"].T
    return z[0], z[1], z[2], z[3], ze


_NC_CACHE = {}


def kernel(**inputs):
    if "nc" not in _NC_CACHE:
        _NC_CACHE["nc"] = build()
    nc = _NC_CACHE["nc"]
    res = run_bass_kernel_spmd(nc, make_in_maps(inputs), list(range(B)))
    return assemble_outputs(res.results)


# revision 10
# speedup vs baseline: 1.0196x; 1.0196x over previous
"""Trainium2 Bass kernel for nn_AttentionD8 (dense transformer attention, D8 irreps).

Sharding: data-parallel over batch B=8 -> 8 NeuronCores, one batch element per
core. No collectives. Each core runs the full per-batch attention block.

Device layout: channel-major ("transposed") everywhere. The host pre-transposes
inputs/weights with numpy and post-transposes outputs, so the device never
transposes anything:
  - qkv projections compute y.T = Wq @ x.T directly (contraction over input
    channels on partitions).
  - attention computes S.T (keys j on partitions, queries i on free axis);
    exp(S.T) is then exactly the stationary operand of the A@V matmul, and an
    extra ones-column appended to V yields the softmax denominators for free.
  - softmax normalization is applied after attention: reciprocal of the
    denominators, PE-broadcast to (96 x 1024) tiles via tiny 0/1 selector
    matmuls (fp32), one elementwise multiply per assembled output tile.
  - biases enter via a ones-row appended to x.T and a bias-row appended to the
    weight (bf16), except the output bias bp_A1 which is added in fp32 on the
    PSUM eviction.
Matmul operands are bf16 (full-rate PE path); all accumulation is fp32 in
PSUM, softmax statistics and normalization are fp32.
"""

import os
import sys

import numpy as np

for _p in ("/opt/trn_rl_repo", os.path.expanduser("~/.axon_site/_ro/trn_rl_repo")):
    if _p not in sys.path and os.path.isdir(_p):
        sys.path.append(_p)

import concourse.bass as bass  # noqa: F401
import concourse.tile as tile
from concourse import bacc, mybir
from concourse.bass_utils import run_bass_kernel_spmd

F32 = mybir.dt.float32
BF16 = mybir.dt.bfloat16
EXP = mybir.ActivationFunctionType.Exp

B, N, H, C = 8, 1024, 12, 96
HD = 64  # head dim
NC2 = (slice(0, 512), slice(512, 1024))  # 512-token chunks


def build():
    nc = bacc.Bacc("TRN2", target_bir_lowering=False, debug=False, num_devices=8)

    def inp(name, shape, dt=BF16):
        return nc.dram_tensor(name, list(shape), dt, kind="ExternalInput").ap()

    def outp(name, shape):
        return nc.dram_tensor(name, list(shape), F32, kind="ExternalOutput").ap()

    xa = [inp("xa1t", (97, N)), inp("xa2t", (96, N)), inp("xb1t", (96, N)),
          inp("xb2t", (96, N))]
    x2 = {(r, k): inp(f"x2t_{r}_{k}", (96, N)) for r in range(2) for k in range(2)}
    wq = [inp("wqa1t", (97, 288)), inp("wqa2t", (96, 288)), inp("wqb1t", (96, 288)),
          inp("wqb2t", (96, 288))]
    wqe = [inp("wqet_0", (96, 576)), inp("wqet_1", (96, 576))]
    wp = [inp("wpa1t", (96, 96)), inp("wpa2t", (96, 96)), inp("wpb1t", (96, 96)),
          inp("wpb2t", (96, 96))]
    wpe = [inp("wpet_0", (96, 192)), inp("wpet_1", (96, 192))]
    eb1d = inp("eb1", (12, 96), F32)
    ebed = inp("ebe", (12, 192), F32)
    bpa1d = inp("bpa1", (96, 1), F32)

    zd = [outp("z1t", (96, N)), outp("z2t", (96, N)), outp("z3t", (96, N)),
          outp("z4t", (96, N))]
    zed = [outp("zet_0", (192, N)), outp("zet_1", (192, N))]

    with tile.TileContext(nc) as tc, \
         tc.tile_pool(name="w", bufs=1) as wpool, \
         tc.tile_pool(name="pack", bufs=1) as packpool:
        # ---- constant / weight loads ----
        wqs = [wpool.tile([t.shape[0], 288], BF16, name=f"wq{i}")
               for i, t in enumerate(wq)]
        wqes = [wpool.tile([96, 576], BF16, name=f"wqe{k}") for k in range(2)]
        wps = [wpool.tile([96, 96], BF16, name=f"wp{i}") for i in range(4)]
        wpes = [wpool.tile([96, 192], BF16, name=f"wpe{k}") for k in range(2)]
        for t, d in zip(wqs + wqes + wps + wpes, wq + wqe + wp + wpe):
            nc.sync.dma_start(t[:], d[:])
        eb1 = wpool.tile([12, 96], F32)
        nc.sync.dma_start(eb1[:], eb1d[:])
        ebe = wpool.tile([12, 192], F32)
        nc.sync.dma_start(ebe[:], ebed[:])
        bpa1 = wpool.tile([96, 1], F32)
        nc.sync.dma_start(bpa1[:], bpa1d[:])

        qpack = [packpool.tile([128, N], BF16, name=f"qp{i}") for i in range(6)]
        kpack = [packpool.tile([128, N], BF16, name=f"kp{i}") for i in range(6)]
        vpack = [packpool.tile([128, 12 * 65], BF16, name=f"vp{i}") for i in range(8)]

        with tc.tile_pool(name="x", bufs=1) as xpool, \
             tc.tile_pool(name="y", bufs=1) as ypool, \
             tc.tile_pool(name="pj", bufs=2, space="PSUM") as pj:
            xs = [xpool.tile([t.shape[0], N], BF16, name=f"x{i}")
                  for i, t in enumerate(xa)]
            for t, d in zip(xs, xa):
                nc.sync.dma_start(t[:], d[:])
            x2s = {}
            for r in range(2):
                for k in range(2):
                    x2s[r, k] = xpool.tile([96, N], BF16, name=f"x2{r}{k}")
                    nc.sync.dma_start(x2s[r, k][:], x2[r, k][:])

            # ---- phase A: q/k projections (channel-major) ----
            yqk = {}
            for bi in range(4):
                for qk in range(2):
                    yt = ypool.tile([96, N], BF16, name=f"y{bi}{qk}")
                    for ic in range(2):
                        ps = pj.tile([96, 512], F32, tag="pj", name="ps_pj")
                        nc.tensor.matmul(
                            ps[:],
                            wqs[bi][:, qk * 96:(qk + 1) * 96],
                            xs[bi][:, NC2[ic]],
                            start=True, stop=True,
                        )
                        nc.vector.tensor_copy(yt[:, NC2[ic]], ps[:])
                    yqk[bi, qk] = yt
            ye = {}
            for r in range(2):
                for oc in range(4):  # q0,q1,k0,k1 96-row chunks of yE
                    yt = ypool.tile([96, N], BF16, name=f"ye{r}{oc}")
                    for ic in range(2):
                        ps = pj.tile([96, 512], F32, tag="pj", name="ps_pj")
                        for k in range(2):
                            nc.tensor.matmul(
                                ps[:],
                                wqes[k][:, oc * 96:(oc + 1) * 96],
                                x2s[r, k][:, NC2[ic]],
                                start=(k == 0), stop=(k == 1),
                            )
                        nc.vector.tensor_copy(yt[:, NC2[ic]], ps[:])
                    ye[r, oc] = yt

            # ---- phase C: v projections, token-major, into packed v tiles ----
            voff = (0, 8, 16, 24)
            for t8 in range(8):
                tok = slice(t8 * 128, (t8 + 1) * 128)
                v3 = vpack[t8].rearrange("p (h c) -> p h c", c=65)
                for bi in range(4):
                    ps = pj.tile([128, 96], F32, tag="pjv", name="ps_pjv")
                    nc.tensor.matmul(
                        ps[:], xs[bi][:, tok], wqs[bi][:, 192:288],
                        start=True, stop=True,
                    )
                    nc.vector.tensor_copy(
                        v3[:, :, voff[bi]:voff[bi] + 8],
                        ps.rearrange("p (h c) -> p h c", c=8),
                    )
                for r in range(2):
                    ps = pj.tile([128, 192], F32, tag="pjv", name="ps_pjv")
                    for k in range(2):
                        nc.tensor.matmul(
                            ps[:], x2s[r, k][:, tok], wqes[k][:, 384:576],
                            start=(k == 0), stop=(k == 1),
                        )
                    nc.vector.tensor_copy(
                        v3[:, :, 32 + r * 16:32 + (r + 1) * 16],
                        ps.rearrange("p (h c) -> p h c", c=16),
                    )
                nc.vector.memset(v3[:, :, 64:65], 1.0)

            # ---- phase B: assemble per-head-pair q/k packs (SBUF->SBUF DMA).
            # Emitted after phase C so C's PE work overlaps these transfers;
            # attention head h only depends on its own pack tiles. ----
            for h in range(H):
                hp, p = divmod(h, 2)
                for qk, pack in ((0, qpack), (1, kpack)):
                    base = p * 64
                    for bi in range(4):
                        nc.sync.dma_start(
                            pack[hp][base + bi * 8: base + (bi + 1) * 8, :],
                            yqk[bi, qk][h * 8:(h + 1) * 8, :],
                        )
                    for r in range(2):
                        oc = qk * 2 + (h // 6)
                        nc.sync.dma_start(
                            pack[hp][base + 32 + r * 16: base + 32 + (r + 1) * 16, :],
                            ye[r, oc][(h % 6) * 16:(h % 6 + 1) * 16, :],
                        )

        # ---- phases D/E: attention, normalize, output projections ----
        with tc.tile_pool(name="o", bufs=1) as opool:
            ots = [opool.tile([96, N], F32, name=f"o{i}t") for i in range(4)]
            oet = {(r, k): opool.tile([96, N], F32, name=f"oet{r}{k}")
                   for r in range(2) for k in range(2)}
            rin = opool.tile([12, N], F32)

            # Software pipeline by one head: head h's QK+exp interleaves with
            # head h-1's AV at j-chunk granularity, so the PE alternates
            # QK/AV matmuls back-to-back (dense PE stream -> HAM stays warm)
            # while ACT's exp of chunk (h, jc) overlaps both.
            with tc.tile_pool(name="es", bufs=16) as espool, \
                 tc.tile_pool(name="avsp", bufs=3) as avspool, \
                 tc.tile_pool(name="st", bufs=2, space="PSUM") as stp, \
                 tc.tile_pool(name="av", bufs=2, space="PSUM") as avp:
                es_prev, av_prev = None, None

                def drain_head(h, es_tiles, avps):
                    avs = avspool.tile([65, N], F32, tag="avs", name="avs")
                    nc.vector.tensor_copy(avs[:], avps[:])
                    for bi in range(4):
                        nc.sync.dma_start(ots[bi][h * 8:(h + 1) * 8, :],
                                          avs[bi * 8:(bi + 1) * 8, :])
                    for r in range(2):
                        nc.sync.dma_start(
                            oet[r, h // 6][(h % 6) * 16:(h % 6 + 1) * 16, :],
                            avs[32 + r * 16:32 + (r + 1) * 16, :],
                        )
                    nc.sync.dma_start(rin[h:h + 1, :], avs[64:65, :])

                for h in range(H + 1):
                    hp, p = divmod(h, 2)
                    base = p * 64
                    es_cur = []
                    avps = (avp.tile([65, N], F32, tag="av", name="ps_av")
                            if h < H else None)
                    for jc in range(8):
                        if h < H:
                            stps = stp.tile([128, N], F32, tag="st", name="ps_st")
                            for ic in range(2):
                                nc.tensor.matmul(
                                    stps[:, NC2[ic]],
                                    kpack[hp][base:base + 64, jc * 128:(jc + 1) * 128],
                                    qpack[hp][base:base + 64, NC2[ic]],
                                    start=True, stop=True,
                                )
                            es = espool.tile([128, N], BF16, tag="es", name="es")
                            nc.scalar.activation(es[:], stps[:], EXP,
                                                 scale=float(HD) ** -0.5)
                            es_cur.append(es)
                        if h > 0:
                            for ic in range(2):
                                nc.tensor.matmul(
                                    av_prev[:, NC2[ic]],
                                    vpack[jc][:, (h - 1) * 65:h * 65],
                                    es_prev[jc][:, NC2[ic]],
                                    start=(jc == 0), stop=(jc == 7),
                                )
                    if h > 0:
                        drain_head(h - 1, es_prev, av_prev)
                    es_prev, av_prev = es_cur, avps

            # ---- phase E: normalize (fp32) + output projections (bf16) ----
            with tc.tile_pool(name="on", bufs=1) as onpool, \
                 tc.tile_pool(name="z", bufs=4) as zpool, \
                 tc.tile_pool(name="ep", bufs=2, space="PSUM") as epp:
                otn = [onpool.tile([96, N], BF16, name=f"on{i}") for i in range(4)]
                oetn = {(r, k): onpool.tile([96, N], BF16, name=f"oen{r}{k}")
                        for r in range(2) for k in range(2)}
                rec = opool.tile([12, N], F32)
                nc.vector.reciprocal_approx_fast(rec[:], rin[:])
                bc1 = epp.tile([96, N], F32, tag="bc", name="ps_bc")
                for ic in range(2):
                    nc.tensor.matmul(bc1[:, NC2[ic]], eb1[:], rec[:, NC2[ic]],
                                     start=True, stop=True)
                for bi in range(4):
                    nc.vector.tensor_mul(otn[bi][:], ots[bi][:], bc1[:])
                for k in range(2):
                    bce = epp.tile([96, N], F32, tag="bc", name="ps_bc")
                    for ic in range(2):
                        nc.tensor.matmul(bce[:, NC2[ic]],
                                         ebe[:, k * 96:(k + 1) * 96],
                                         rec[:, NC2[ic]], start=True, stop=True)
                    for r in range(2):
                        nc.vector.tensor_mul(oetn[r, k][:], oet[r, k][:], bce[:])

                for zi in range(4):
                    zs = zpool.tile([96, N], F32, tag="z", name="zs")
                    for ic in range(2):
                        ps = epp.tile([96, 512], F32, tag="zps", name="ps_z")
                        nc.tensor.matmul(ps[:], wps[zi][:], otn[zi][:, NC2[ic]],
                                         start=True, stop=True)
                        if zi == 0:
                            nc.vector.tensor_scalar_add(zs[:, NC2[ic]], ps[:],
                                                        bpa1[:])
                        else:
                            nc.vector.tensor_copy(zs[:, NC2[ic]], ps[:])
                    nc.sync.dma_start(zd[zi][:], zs[:])
                for r in range(2):
                    for mc in range(2):
                        zs = zpool.tile([96, N], F32, tag="z", name="zs")
                        for ic in range(2):
                            ps = epp.tile([96, 512], F32, tag="zps", name="ps_z")
                            for k in range(2):
                                nc.tensor.matmul(
                                    ps[:], wpes[k][:, mc * 96:(mc + 1) * 96],
                                    oetn[r, k][:, NC2[ic]],
                                    start=(k == 0), stop=(k == 1),
                                )
                            nc.vector.tensor_copy(zs[:, NC2[ic]], ps[:])
                        nc.sync.dma_start(zed[r][mc * 96:(mc + 1) * 96, :], zs[:])

    nc.compile()
    return nc


def make_in_maps(inputs):
    from ml_dtypes import bfloat16

    b16 = lambda a: np.ascontiguousarray(np.asarray(a, dtype=np.float32)).astype(bfloat16)  # noqa: E731
    f32c = lambda a: np.ascontiguousarray(a, dtype=np.float32)  # noqa: E731
    ones = np.ones((1, N), np.float32)
    shared = {
        "wqa1t": b16(np.concatenate(
            [np.asarray(inputs["wq_A1"], np.float32).T,
             np.asarray(inputs["bq_A1"], np.float32)[None, :]], 0)),
        "wqa2t": b16(np.asarray(inputs["wq_A2"]).T),
        "wqb1t": b16(np.asarray(inputs["wq_B1"]).T),
        "wqb2t": b16(np.asarray(inputs["wq_B2"]).T),
        "wpa1t": b16(np.asarray(inputs["wp_A1"]).T),
        "wpa2t": b16(np.asarray(inputs["wp_A2"]).T),
        "wpb1t": b16(np.asarray(inputs["wp_B1"]).T),
        "wpb2t": b16(np.asarray(inputs["wp_B2"]).T),
        "eb1": (np.arange(96)[None, :] // 8 == np.arange(12)[:, None]).astype(np.float32),
        "ebe": (np.arange(192)[None, :] // 16 == np.arange(12)[:, None]).astype(np.float32),
        "bpa1": f32c(np.asarray(inputs["bp_A1"], np.float32)[:, None]),
    }
    for k in range(2):
        shared[f"wqet_{k}"] = b16(np.asarray(inputs["wq_E"]).T[k * 96:(k + 1) * 96])
        shared[f"wpet_{k}"] = b16(np.asarray(inputs["wp_E"]).T[k * 96:(k + 1) * 96])
    maps = []
    for b in range(B):
        m = dict(shared)
        m["xa1t"] = b16(np.concatenate(
            [np.asarray(inputs["x_A1"][b], np.float32).T, ones], 0))
        m["xa2t"] = b16(np.asarray(inputs["x_A2"][b]).T)
        m["xb1t"] = b16(np.asarray(inputs["x_B1"][b]).T)
        m["xb2t"] = b16(np.asarray(inputs["x_B2"][b]).T)
        for r in range(2):
            for k in range(2):
                m[f"x2t_{r}_{k}"] = b16(
                    np.asarray(inputs["x_2d"][b, :, r, k * 96:(k + 1) * 96]).T)
        maps.append(m)
    return maps


def assemble_outputs(results):
    z = [np.empty((B, N, 96), np.float32) for _ in range(4)]
    ze = np.empty((B, N, 2, 192), np.float32)
    for b in range(B):
        for i in range(4):
            z[i][b] = results[b][f"z{i + 1}t"].T
        for r in range(2):
            ze[b, :, r, :] = results[b][f"zet_{r}"].T
    return z[0], z[1], z[2], z[3], ze


_NC_CACHE = {}


def kernel(**inputs):
    if "nc" not in _NC_CACHE:
        _NC_CACHE["nc"] = build()
    nc = _NC_CACHE["nc"]
    res = run_bass_kernel_spmd(nc, make_in_maps(inputs), list(range(B)))
    return assemble_outputs(res.results)


# revision 12
# speedup vs baseline: 1.1881x; 1.1653x over previous
"""Trainium2 Bass kernel for nn_AttentionD8 (dense transformer attention, D8 irreps).

Sharding: data-parallel over batch B=8 -> 8 NeuronCores, one batch element per
core. No collectives. Each core runs the full per-batch attention block.

Device layout: channel-major ("transposed") everywhere; the host pre-transposes
inputs/weights with numpy and post-transposes outputs, so the device never
transposes anything:
  - qkv projections compute y.T = Wq @ x.T directly.
  - attention computes S.T (keys j on partitions, queries i on free axis);
    exp(S.T) is exactly the stationary operand of the A@V matmul, and an extra
    ones-column appended to V yields the softmax denominators for free.
  - normalization happens after attention: reciprocal of the denominators,
    PE-broadcast to (96 x N) tiles via 0/1 selector matmuls, one elementwise
    multiply per assembled output tile.
  - biases enter via a ones-row appended to x.T and a bias-row on the weight
    (bf16), except bp_A1 which is added in fp32 on the PSUM eviction.

Within-head channel orders are chosen so every partition-interleaving
SBUF->SBUF assembly is ONE dma whose flat element stream matches on both
sides:
  - q/k packs: 1d rows d*4+bi, 2d rows 32+e*2+r (a contraction-order
    permutation, consistent between q and k, so S is unchanged).
  - V-pack columns use the same order, so the attention-output rows stream
    straight into the merged o-tiles.

The PE clock-gate (HAM) has hysteresis: the attention QK/AV interleave is
~99% busy with micro-holes and never *transitions* the gate, so whatever
state it enters with persists. Dense same-weight warmup bursts are issued at
kernel start and right before attention to enter warm (2.4 GHz).

Matmul operands are bf16; accumulation is fp32 in PSUM; softmax statistics
and normalization are fp32.
"""

import os
import sys

import numpy as np

for _p in ("/opt/trn_rl_repo", os.path.expanduser("~/.axon_site/_ro/trn_rl_repo")):
    if _p not in sys.path and os.path.isdir(_p):
        sys.path.append(_p)

import concourse.bass as bass  # noqa: F401
import concourse.tile as tile
from concourse import bacc, mybir
from concourse.bass_utils import run_bass_kernel_spmd

F32 = mybir.dt.float32
BF16 = mybir.dt.bfloat16
EXP = mybir.ActivationFunctionType.Exp

B, N, H, C = 8, 1024, 12, 96
HD = 64
NC2 = (slice(0, 512), slice(512, 1024))


def build():
    nc = bacc.Bacc("TRN2", target_bir_lowering=False, debug=False, num_devices=8)

    def inp(name, shape, dt=BF16):
        return nc.dram_tensor(name, list(shape), dt, kind="ExternalInput").ap()

    def outp(name, shape):
        return nc.dram_tensor(name, list(shape), F32, kind="ExternalOutput").ap()

    xa = [inp("xa1t", (97, N)), inp("xa2t", (96, N)), inp("xb1t", (96, N)),
          inp("xb2t", (96, N))]
    x2 = {(r, k): inp(f"x2t_{r}_{k}", (96, N)) for r in range(2) for k in range(2)}
    wq = [inp("wqa1t", (97, 288)), inp("wqa2t", (96, 288)), inp("wqb1t", (96, 288)),
          inp("wqb2t", (96, 288))]
    wqe = [inp("wqet_0", (96, 576)), inp("wqet_1", (96, 576))]
    wp = [inp("wpa1t", (96, 96)), inp("wpa2t", (96, 96)), inp("wpb1t", (96, 96)),
          inp("wpb2t", (96, 96))]
    wpe = [inp("wpet_0", (96, 192)), inp("wpet_1", (96, 192))]
    eb1d = inp("eb1", (12, 96), F32)
    ebed = inp("ebe", (12, 192), F32)
    bpa1d = inp("bpa1", (96, 1), F32)

    zd = [outp("z1t", (96, N)), outp("z2t", (96, N)), outp("z3t", (96, N)),
          outp("z4t", (96, N))]
    zed = [outp("zet_0", (192, N)), outp("zet_1", (192, N))]

    with tile.TileContext(nc) as tc, \
         tc.tile_pool(name="w", bufs=1) as wpool, \
         tc.tile_pool(name="pack", bufs=1) as packpool:
        # ---- constant / weight loads (wqe0 first: it feeds the warmup) ----
        wqes = [wpool.tile([96, 576], BF16, name=f"wqe{k}") for k in range(2)]
        wqs = [wpool.tile([t.shape[0], 288], BF16, name=f"wq{i}")
               for i, t in enumerate(wq)]
        wps = [wpool.tile([96, 96], BF16, name=f"wp{i}") for i in range(4)]
        wpes = [wpool.tile([96, 192], BF16, name=f"wpe{k}") for k in range(2)]
        for t, d in zip(wqes + wqs + wps + wpes, wqe + wq + wp + wpe):
            nc.sync.dma_start(t[:], d[:])
        eb1 = wpool.tile([12, 96], F32)
        nc.sync.dma_start(eb1[:], eb1d[:])
        ebe = wpool.tile([12, 192], F32)
        nc.sync.dma_start(ebe[:], ebed[:])
        bpa1 = wpool.tile([96, 1], F32)
        nc.sync.dma_start(bpa1[:], bpa1d[:])

        qpack = [packpool.tile([128, N], BF16, name=f"qp{i}") for i in range(6)]
        kpack = [packpool.tile([128, N], BF16, name=f"kp{i}") for i in range(6)]
        vpack = [packpool.tile([128, 12 * 65], BF16, name=f"vp{i}") for i in range(8)]

        with tc.tile_pool(name="x", bufs=1) as xpool, \
             tc.tile_pool(name="y", bufs=1) as ypool, \
             tc.tile_pool(name="pj", bufs=2, space="PSUM") as pj:

            # ---- PE warmup: dense same-weight matmuls flip HAM to 2.4 GHz ----
            for i in range(25):
                wu = pj.tile([96, 512], F32, tag="pj", name="wu")
                nc.tensor.matmul(wu[:], wqes[0][:, :96], wqes[0][:, :512],
                                 start=True, stop=True)

            xs = [xpool.tile([t.shape[0], N], BF16, name=f"x{i}")
                  for i, t in enumerate(xa)]
            for t, d in zip(xs, xa):
                nc.sync.dma_start(t[:], d[:])
            x2s = {}
            for r in range(2):
                for k in range(2):
                    x2s[r, k] = xpool.tile([96, N], BF16, name=f"x2{r}{k}")
                    nc.sync.dma_start(x2s[r, k][:], x2[r, k][:])

            # y layout: 1d merged per qk -> (96 rows h*8+d) x (4 irreps x N);
            # 2d merged per (qk, head-half) -> (96 rows (h%6)*16+e) x (2 r x N)
            y1d = [ypool.tile([96, 4 * N], BF16, name=f"y1d{qk}") for qk in range(2)]
            y2d = {(qk, hh): ypool.tile([96, 2 * N], BF16, name=f"y2d{qk}{hh}")
                   for qk in range(2) for hh in range(2)}

            # ---- phase A: q/k projections ----
            for bi in range(4):
                for qk in range(2):
                    for ic in range(2):
                        ps = pj.tile([96, 512], F32, tag="pj", name="ps_pj")
                        nc.tensor.matmul(
                            ps[:],
                            wqs[bi][:, qk * 96:(qk + 1) * 96],
                            xs[bi][:, NC2[ic]],
                            start=True, stop=True,
                        )
                        nc.vector.tensor_copy(
                            y1d[qk][:, bi * N + ic * 512: bi * N + ic * 512 + 512],
                            ps[:])
            for r in range(2):
                for oc in range(4):  # oc = qk*2 + hh
                    qk, hh = divmod(oc, 2)
                    for ic in range(2):
                        ps = pj.tile([96, 512], F32, tag="pj", name="ps_pj")
                        for k in range(2):
                            nc.tensor.matmul(
                                ps[:],
                                wqes[k][:, oc * 96:(oc + 1) * 96],
                                x2s[r, k][:, NC2[ic]],
                                start=(k == 0), stop=(k == 1),
                            )
                        nc.vector.tensor_copy(
                            y2d[qk, hh][:, r * N + ic * 512: r * N + ic * 512 + 512],
                            ps[:])

            # ---- phase C: v projections, token-major, into packed v tiles ----
            # vpack column order per head: [1d: d*4+bi | 2d: 32+e*2+r | ones]
            for t8 in range(8):
                tok = slice(t8 * 128, (t8 + 1) * 128)
                v3 = vpack[t8].rearrange("p (h c) -> p h c", c=65)
                v1dv = v3[:, :, 0:32].rearrange("p h (d b) -> p h d b", b=4)
                v2dv = v3[:, :, 32:64].rearrange("p h (e r) -> p h e r", r=2)
                for bi in range(4):
                    ps = pj.tile([128, 96], F32, tag="pjv", name="ps_pjv")
                    nc.tensor.matmul(
                        ps[:], xs[bi][:, tok], wqs[bi][:, 192:288],
                        start=True, stop=True,
                    )
                    nc.vector.tensor_copy(
                        v1dv[:, :, :, bi],
                        ps.rearrange("p (h d) -> p h d", d=8),
                    )
                for r in range(2):
                    ps = pj.tile([128, 192], F32, tag="pjv", name="ps_pjv")
                    for k in range(2):
                        nc.tensor.matmul(
                            ps[:], x2s[r, k][:, tok], wqes[k][:, 384:576],
                            start=(k == 0), stop=(k == 1),
                        )
                    nc.vector.tensor_copy(
                        v2dv[:, :, :, r],
                        ps.rearrange("p (h e) -> p h e", e=16),
                    )
                nc.vector.memset(v3[:, :, 64:65], 1.0)

            # ---- phase B: assemble q/k packs. One dma per (head, qk, 1d/2d);
            # flat element streams match (dst is a plain 2D partition slice).
            # Issue alternates sync/gpsimd queues. ----
            for h in range(H):
                hp, p = divmod(h, 2)
                eng = [nc.sync, nc.gpsimd][h % 2]
                for qk, pack in ((0, qpack), (1, kpack)):
                    base = p * 64
                    eng.dma_start(
                        pack[hp][base: base + 32, :],
                        y1d[qk][h * 8:(h + 1) * 8, :].rearrange(
                            "d (b t) -> d b t", b=4),
                    )
                    eng.dma_start(
                        pack[hp][base + 32: base + 64, :],
                        y2d[qk, h // 6][(h % 6) * 16:(h % 6 + 1) * 16, :].rearrange(
                            "e (r t) -> e r t", r=2),
                    )

        # ---- phases D/E: attention, normalize, output projections ----
        with tc.tile_pool(name="o", bufs=1) as opool:
            # merged assembled outputs: o1d rows h*8+d, free (bi, t);
            # oe[kc] rows (h%6)*16+e, free (r, t)
            o1d = opool.tile([96, 4 * N], F32, name="o1d")
            oe = [opool.tile([96, 2 * N], F32, name=f"oe{kc}") for kc in range(2)]
            rin = opool.tile([12, N], F32)

            with tc.tile_pool(name="es", bufs=16) as espool, \
                 tc.tile_pool(name="avsp", bufs=3) as avspool, \
                 tc.tile_pool(name="st", bufs=2, space="PSUM") as stp, \
                 tc.tile_pool(name="av", bufs=2, space="PSUM") as avp:

                # bridge warmup: keep/restore HAM warm across the assembly gap
                for i in range(15):
                    wu = stp.tile([96, 512], F32, tag="st", name="wu2")
                    nc.tensor.matmul(wu[:], wqes[0][:, :96], wqes[0][:, :512],
                                     start=True, stop=True)

                def drain_head(h, avps):
                    avs = avspool.tile([65, N], F32, tag="avs", name="avs")
                    nc.vector.tensor_copy(avs[:], avps[:])
                    nc.gpsimd.dma_start(o1d[h * 8:(h + 1) * 8, :], avs[0:32, :])
                    nc.gpsimd.dma_start(
                        oe[h // 6][(h % 6) * 16:(h % 6 + 1) * 16, :], avs[32:64, :])
                    nc.gpsimd.dma_start(rin[h:h + 1, :], avs[64:65, :])

                es_prev, av_prev = None, None
                for h in range(H + 1):
                    hp, p = divmod(h, 2)
                    base = p * 64
                    es_cur = []
                    avps = (avp.tile([65, N], F32, tag="av", name="ps_av")
                            if h < H else None)
                    for jc in range(8):
                        if h < H:
                            stps = stp.tile([128, N], F32, tag="st", name="ps_st")
                            for ic in range(2):
                                nc.tensor.matmul(
                                    stps[:, NC2[ic]],
                                    kpack[hp][base:base + 64, jc * 128:(jc + 1) * 128],
                                    qpack[hp][base:base + 64, NC2[ic]],
                                    start=True, stop=True,
                                )
                            es = espool.tile([128, N], BF16, tag="es", name="es")
                            nc.scalar.activation(es[:], stps[:], EXP,
                                                 scale=float(HD) ** -0.5)
                            es_cur.append(es)
                        if h > 0:
                            for ic in range(2):
                                nc.tensor.matmul(
                                    av_prev[:, NC2[ic]],
                                    vpack[jc][:, (h - 1) * 65:h * 65],
                                    es_prev[jc][:, NC2[ic]],
                                    start=(jc == 0), stop=(jc == 7),
                                )
                    if h > 0:
                        drain_head(h - 1, av_prev)
                    es_prev, av_prev = es_cur, avps

            # ---- phase E: normalize (fp32) + output projections (bf16) ----
            with tc.tile_pool(name="on", bufs=1) as onpool, \
                 tc.tile_pool(name="z", bufs=4) as zpool, \
                 tc.tile_pool(name="ep", bufs=2, space="PSUM") as epp:
                otn = [onpool.tile([96, N], BF16, name=f"on{i}") for i in range(4)]
                oetn = {(r, k): onpool.tile([96, N], BF16, name=f"oen{r}{k}")
                        for r in range(2) for k in range(2)}
                rec = opool.tile([12, N], F32)
                nc.vector.reciprocal_approx_fast(rec[:], rin[:])
                bc1 = epp.tile([96, N], F32, tag="bc", name="ps_bc")
                for ic in range(2):
                    nc.tensor.matmul(bc1[:, NC2[ic]], eb1[:], rec[:, NC2[ic]],
                                     start=True, stop=True)
                for bi in range(4):
                    nc.vector.tensor_mul(otn[bi][:], o1d[:, bi * N:(bi + 1) * N],
                                         bc1[:])
                for k in range(2):
                    bce = epp.tile([96, N], F32, tag="bc", name="ps_bc")
                    for ic in range(2):
                        nc.tensor.matmul(bce[:, NC2[ic]],
                                         ebe[:, k * 96:(k + 1) * 96],
                                         rec[:, NC2[ic]], start=True, stop=True)
                    for r in range(2):
                        nc.vector.tensor_mul(oetn[r, k][:],
                                             oe[k][:, r * N:(r + 1) * N], bce[:])

                for zi in range(4):
                    zs = zpool.tile([96, N], F32, tag="z", name="zs")
                    for ic in range(2):
                        ps = epp.tile([96, 512], F32, tag="zps", name="ps_z")
                        nc.tensor.matmul(ps[:], wps[zi][:], otn[zi][:, NC2[ic]],
                                         start=True, stop=True)
                        if zi == 0:
                            nc.vector.tensor_scalar_add(zs[:, NC2[ic]], ps[:],
                                                        bpa1[:])
                        else:
                            nc.vector.tensor_copy(zs[:, NC2[ic]], ps[:])
                    nc.sync.dma_start(zd[zi][:], zs[:])
                for r in range(2):
                    for mc in range(2):
                        zs = zpool.tile([96, N], F32, tag="z", name="zs")
                        for ic in range(2):
                            ps = epp.tile([96, 512], F32, tag="zps", name="ps_z")
                            for k in range(2):
                                nc.tensor.matmul(
                                    ps[:], wpes[k][:, mc * 96:(mc + 1) * 96],
                                    oetn[r, k][:, NC2[ic]],
                                    start=(k == 0), stop=(k == 1),
                                )
                            nc.vector.tensor_copy(zs[:, NC2[ic]], ps[:])
                        nc.sync.dma_start(zed[r][mc * 96:(mc + 1) * 96, :], zs[:])

    nc.compile()
    return nc


def make_in_maps(inputs):
    from ml_dtypes import bfloat16

    b16 = lambda a: np.ascontiguousarray(np.asarray(a, dtype=np.float32)).astype(bfloat16)  # noqa: E731
    f32c = lambda a: np.ascontiguousarray(a, dtype=np.float32)  # noqa: E731
    ones = np.ones((1, N), np.float32)
    shared = {
        "wqa1t": b16(np.concatenate(
            [np.asarray(inputs["wq_A1"], np.float32).T,
             np.asarray(inputs["bq_A1"], np.float32)[None, :]], 0)),
        "wqa2t": b16(np.asarray(inputs["wq_A2"]).T),
        "wqb1t": b16(np.asarray(inputs["wq_B1"]).T),
        "wqb2t": b16(np.asarray(inputs["wq_B2"]).T),
        "wpa1t": b16(np.asarray(inputs["wp_A1"]).T),
        "wpa2t": b16(np.asarray(inputs["wp_A2"]).T),
        "wpb1t": b16(np.asarray(inputs["wp_B1"]).T),
        "wpb2t": b16(np.asarray(inputs["wp_B2"]).T),
        "eb1": (np.arange(96)[None, :] // 8 == np.arange(12)[:, None]).astype(np.float32),
        "ebe": (np.arange(192)[None, :] // 16 == np.arange(12)[:, None]).astype(np.float32),
        "bpa1": f32c(np.asarray(inputs["bp_A1"], np.float32)[:, None]),
    }
    for k in range(2):
        shared[f"wqet_{k}"] = b16(np.asarray(inputs["wq_E"]).T[k * 96:(k + 1) * 96])
        shared[f"wpet_{k}"] = b16(np.asarray(inputs["wp_E"]).T[k * 96:(k + 1) * 96])
    maps = []
    for b in range(B):
        m = dict(shared)
        m["xa1t"] = b16(np.concatenate(
            [np.asarray(inputs["x_A1"][b], np.float32).T, ones], 0))
        m["xa2t"] = b16(np.asarray(inputs["x_A2"][b]).T)
        m["xb1t"] = b16(np.asarray(inputs["x_B1"][b]).T)
        m["xb2t"] = b16(np.asarray(inputs["x_B2"][b]).T)
        for r in range(2):
            for k in range(2):
                m[f"x2t_{r}_{k}"] = b16(
                    np.asarray(inputs["x_2d"][b, :, r, k * 96:(k + 1) * 96]).T)
        maps.append(m)
    return maps


def assemble_outputs(results):
    z = [np.empty((B, N, 96), np.float32) for _ in range(4)]
    ze = np.empty((B, N, 2, 192), np.float32)
    for b in range(B):
        for i in range(4):
            z[i][b] = results[b][f"z{i + 1}t"].T
        for r in range(2):
            ze[b, :, r, :] = results[b][f"zet_{r}"].T
    return z[0], z[1], z[2], z[3], ze


_NC_CACHE = {}


def kernel(**inputs):
    if "nc" not in _NC_CACHE:
        _NC_CACHE["nc"] = build()
    nc = _NC_CACHE["nc"]
    res = run_bass_kernel_spmd(nc, make_in_maps(inputs), list(range(B)))
    return assemble_outputs(res.results)
